# revision 4
# baseline (speedup 1.0000x reference)
"""Trainium2 Bass kernel for nn_AbstractODEDecoder.

Data-parallel over batch across 8 cores. Each core:
  - integrates the 100-step dopri5 ODE for its 64-row batch shard
    (fp32, feature-major, fully unrolled),
  - spills v_all rows to DRAM batch-major per step,
  - decodes 12800 (b, n) rows sorted by time index, gathering
    latent+z_static rows via dma_gather (float32r matmuls, N=400),
  - interleaved with the ODE via AP-granular DRAM deps.
Host: unique-time grid, per-stage effective biases, gather indices,
input sharding / output assembly.
"""
import numpy as np

import concourse.bass as bass
import concourse.mybir as mybir
import concourse.tile as tile
from concourse import bacc
from concourse.masks import make_identity

F32 = mybir.dt.float32
F32R = mybir.dt.float32r
I16 = mybir.dt.int16
TANH = mybir.ActivationFunctionType.Tanh
RELU = mybir.ActivationFunctionType.Relu
ADD = mybir.AluOpType.add
MULT = mybir.AluOpType.mult

B, N, ZD, LD, HD, T = 512, 200, 256, 128, 512, 100
NST, NCORE, BL = 100, 8, 64
ROWS, RT, NTILE = BL * N, 400, BL * N // 400
NH = 1                    # batch-halves pipelined per core
NB = BL // NH             # batch per half

C_ = [0.0, 1.0 / 5.0, 3.0 / 10.0, 4.0 / 5.0, 8.0 / 9.0, 1.0]
A_ = [[],
      [1.0 / 5.0],
      [3.0 / 40.0, 9.0 / 40.0],
      [44.0 / 45.0, -56.0 / 15.0, 32.0 / 9.0],
      [19372.0 / 6561.0, -25360.0 / 2187.0, 64448.0 / 6561.0, -212.0 / 729.0],
      [9017.0 / 3168.0, -355.0 / 33.0, 46732.0 / 5247.0, 49.0 / 176.0,
       -5103.0 / 18656.0]]
B_ = [35.0 / 384.0, 0.0, 500.0 / 1113.0, 125.0 / 192.0, -2187.0 / 6784.0,
      11.0 / 84.0]

_BUILD_CACHE = {}


def _build(maxinds, dts):
    """Build + bacc-compile the SPMD program. maxinds: per-decode-tile max
    time index (same for every core by construction of the schedule)."""
    key = ('v1', NH, tuple(maxinds), tuple(np.float32(dts).tolist()))
    if key in _BUILD_CACHE:
        return _BUILD_CACHE[key]

    nc = bacc.Bacc('TRN2', target_bir_lowering=False, debug=False,
                   num_devices=NCORE, num_swdge_queues=4)
    dram = {}

    def din(name, shape, dt):
        dram[name] = nc.dram_tensor(name, shape, dt, kind='ExternalInput').ap()
        return dram[name]

    # ODE weights (fp32) in lhsT tile layouts
    w1l_d = din('w1l', [128, 4, 128], F32)        # [p, m, f] = W1[p, m*128+f]
    w2_d = din('w2', [128, 4, 4, 128], F32)       # [p, k, m, f]
    w3_d = din('w3', [128, 4, 128], F32)          # [p, k, f]
    b1res_d = din('b1res', [128, NST * 6, 4], F32)
    b2rep_d = din('b2rep', [128, 4], F32)
    db3_d = din('db3', [128, NST], F32)
    c1_d = din('c1', [128, 4, BL], F32)
    vlt0_d = din('vlt0', [128, BL], F32)
    zl_d = din('zl', [BL, 128], F32)              # z latent, batch-major
    zz_d = din('zz', [BL, 128], F32)              # z static, batch-major
    # decode weights (float32r)
    d1l_d = din('d1l', [128, 4, 128], F32R)
    d1z_d = din('d1z', [128, 4, 128], F32R)
    d1x_d = din('d1x', [1, 512], F32R)
    d2_d = din('d2', [128, 4, 4, 128], F32R)
    d3_d = din('d3', [128, 4, 4, 128], F32R)
    dbias_d = din('dbias', [128, 4, 3], F32)
    xs_d = din('xs', [1, ROWS], F32R)
    gidx_d = din('gidx', [128, NTILE * 32], I16)  # 32 idx-cols per tile (400/16=25, pad 32)

    vall_d = nc.dram_tensor('vall', [(NST + 1) * BL, 256], F32).ap()
    out_d = nc.dram_tensor('outT', [4, 128, ROWS], F32,
                           kind='ExternalOutput').ap()

    with tile.TileContext(nc) as tc, \
         tc.tile_pool(name='sing', bufs=1) as sing, \
         tc.tile_pool(name='ode', bufs=3) as ode, \
         tc.tile_pool(name='dec', bufs=2) as dec, \
         tc.tile_pool(name='psA', bufs=1, space='PSUM') as psA, \
         tc.tile_pool(name='psD', bufs=2, space='PSUM') as psD:

        # ---- residents ----
        w1l = sing.tile([128, 4, 128], F32)
        nc.sync.dma_start(out=w1l, in_=w1l_d)
        w2 = sing.tile([128, 4, 4, 128], F32)
        nc.sync.dma_start(out=w2, in_=w2_d)
        w3 = sing.tile([128, 4, 128], F32)
        nc.sync.dma_start(out=w3, in_=w3_d)
        b1res = sing.tile([128, NST * 6, 4], F32)
        nc.sync.dma_start(out=b1res, in_=b1res_d)
        b2rep = sing.tile([128, 4], F32)
        nc.sync.dma_start(out=b2rep, in_=b2rep_d)
        db3 = sing.tile([128, NST], F32)
        nc.sync.dma_start(out=db3, in_=db3_d)
        c1 = sing.tile([128, 4, BL], F32)
        nc.sync.dma_start(out=c1, in_=c1_d)
        d1l = sing.tile([128, 4, 128], F32R)
        nc.sync.dma_start(out=d1l, in_=d1l_d)
        d1z = sing.tile([128, 4, 128], F32R)
        nc.sync.dma_start(out=d1z, in_=d1z_d)
        d1x = sing.tile([1, 512], F32R)
        nc.sync.dma_start(out=d1x, in_=d1x_d)
        d2 = sing.tile([128, 4, 4, 128], F32R)
        nc.sync.dma_start(out=d2, in_=d2_d)
        d3 = sing.tile([128, 4, 4, 128], F32R)
        nc.sync.dma_start(out=d3, in_=d3_d)
        dbias = sing.tile([128, 4, 3], F32)
        nc.sync.dma_start(out=dbias, in_=dbias_d)
        xs = sing.tile([1, ROWS], F32R)
        nc.sync.dma_start(out=xs, in_=xs_d)
        gidx = sing.tile([128, NTILE * 32], I16)
        nc.sync.dma_start(out=gidx, in_=gidx_d)
        ident = sing.tile([128, 128], F32)
        make_identity(nc, ident)

        # ---- v_all init: z_static for every step; latent for step 0 ----
        vall_v = vall_d.rearrange('(s b) f -> s b f', b=BL)
        zz_b = bass.AP(tensor=zz_d.tensor, offset=zz_d.offset,
                       ap=[[0, NST + 1], *zz_d.ap])
        nc.sync.dma_start(out=vall_v[:, :, 128:256], in_=zz_b)
        nc.sync.dma_start(out=vall_v[0, :, 0:128], in_=zl_d)

        # ---- initial v latent per half ----
        vl = []
        for h in range(NH):
            t0 = ode.tile([128, NB], F32, tag=f'vl{h}', name=f'vl0_{h}')
            nc.sync.dma_start(out=t0, in_=vlt0_d[:, h * NB:(h + 1) * NB])
            vl.append(t0)

        def bcast(ap, n):
            return bass.AP(tensor=ap.tensor, offset=ap.offset,
                           ap=[*ap.ap, [0, n]])

        # per-half psum tiles for this step (allocated per step below)
        def emit_stage(h, s, i, h1kb, h2p, kb_ap):
            si = s * 6 + i
            # vtmp for this stage
            if i == 0:
                vt = vl[h]
            else:
                vt = ode.tile([128, NB], F32, tag=f'vt{h}', name=f'vt_{h}_{si}')
                first = True
                dt_s = dts[s]
                for j, a in enumerate(A_[i]):
                    if a == 0.0:
                        continue
                    coef = float(np.float32(dt_s) * np.float32(a))
                    if first:
                        nc.vector.tensor_scalar(vt, kb_ap(j), coef, None, MULT)
                        nc.vector.tensor_tensor(vt, vt, vl[h], ADD)
                        first = False
                    else:
                        tmp = ode.tile([128, NB], F32, tag=f'tm{h}',
                                       name=f'tm_{h}_{si}_{j}')
                        nc.vector.tensor_scalar(tmp, kb_ap(j), coef, None, MULT)
                        nc.vector.tensor_tensor(vt, vt, tmp, ADD)
            # L1: 4 matmuls K=128 (vL part only) into h1 region
            for m in range(4):
                nc.tensor.matmul(h1kb[:, m * NB:(m + 1) * NB],
                                 w1l[:, m, :], vt, start=True, stop=True)
            # bias: cb = c1slice + b1eff broadcast (off-chain), add on-chain
            cb = ode.tile([128, 4, NB], F32, tag=f'cb{h}', name=f'cb_{h}_{si}')
            nc.vector.tensor_tensor(
                cb, c1[:, :, h * NB:(h + 1) * NB],
                bcast(b1res[:, si, :], NB), ADD)
            h1v = h1kb[:, 0:4 * NB].rearrange('p (m j) -> p m j', m=4)
            nc.vector.tensor_tensor(h1v, h1v, cb, ADD)
            h1t = ode.tile([128, 4, NB], F32, tag=f'h1t{h}', name=f'h1t_{h}_{si}')
            nc.scalar.activation(h1t, h1v, TANH)
            # L2: 16 matmuls K=512
            for m in range(4):
                for k in range(4):
                    nc.tensor.matmul(h2p[:, m * NB:(m + 1) * NB],
                                     w2[:, k, m, :], h1t[:, k, :],
                                     start=(k == 0), stop=(k == 3))
            h2v = h2p[:, 0:4 * NB].rearrange('p (m j) -> p m j', m=4)
            nc.vector.tensor_tensor(h2v, h2v, bcast(b2rep, NB), ADD)
            h2t = ode.tile([128, 4, NB], F32, tag=f'h2t{h}', name=f'h2t_{h}_{si}')
            nc.scalar.activation(h2t, h2v, TANH)
            # L3: 4 matmuls K=512 -> ktilde slot i
            for k in range(4):
                nc.tensor.matmul(kb_ap(i), w3[:, k, :], h2t[:, k, :],
                                 start=(k == 0), stop=(k == 3))

        def emit_vupdate(h, s, kb_ap):
            dt_s = dts[s]
            acc = ode.tile([128, NB], F32, tag=f'ac{h}', name=f'ac_{h}_{s}')
            first = True
            for j, b in enumerate(B_):
                if b == 0.0:
                    continue
                coef = float(np.float32(dt_s) * np.float32(b))
                if first:
                    nc.vector.tensor_scalar(acc, kb_ap(j), coef, None, MULT)
                    first = False
                else:
                    tmp = ode.tile([128, NB], F32, tag=f'tm{h}',
                                   name=f'tmu_{h}_{s}_{j}')
                    nc.vector.tensor_scalar(tmp, kb_ap(j), coef, None, MULT)
                    nc.vector.tensor_tensor(acc, acc, tmp, ADD)
            vnew = ode.tile([128, NB], F32, tag=f'vl{h}', name=f'vn_{h}_{s}')
            nc.vector.tensor_tensor(vnew, vl[h], acc, ADD)
            nc.vector.tensor_scalar(vnew, vnew, db3[:, s:s + 1], None, ADD)
            vl[h] = vnew
            # spill v_all rows (s+1): transpose to batch-major and DMA
            trp = psA.tile([128, 128], F32, tag='tr', name=f'vtr_{h}_{s}')
            nc.tensor.transpose(trp[0:NB, :], vnew, ident)
            vvb = ode.tile([NB, 128], F32, tag=f'vb{h}', name=f'vb_{h}_{s}')
            nc.vector.tensor_copy(vvb, trp[0:NB, :])
            nc.sync.dma_start(
                out=vall_v[s + 1, h * NB:(h + 1) * NB, 0:128], in_=vvb)

        def emit_decode_tile(ti, mi):
            r0 = ti * RT
            # gather 400 rows of 256 f32 from the written prefix of vall
            g_sb = dec.tile([128, 4, 256], F32, tag='g', name=f'g_{ti}')
            nc.gpsimd.dma_gather(
                g_sb[:], vall_d[0:(mi + 1) * BL], gidx[:, ti * 32:ti * 32 + 25],
                num_idxs=RT, num_idxs_reg=RT, elem_size=256,
                queue_num=ti % 4)
            # transpose to feature-major: latent rows then z rows
            latT = dec.tile([128, RT], F32R, tag='latT', name=f'latT_{ti}')
            zT = dec.tile([128, RT], F32R, tag='zT', name=f'zT_{ti}')
            for half, dst in ((0, latT), (1, zT)):
                trp = psA.tile([128, 512], F32, tag='tr', name=f'dtr_{ti}_{half}')
                for c in range(4):
                    nc.tensor.transpose(
                        trp[:, c * 128:(c + 1) * 128],
                        g_sb[:, c, half * 128:(half + 1) * 128], ident)
                nc.vector.tensor_copy(dst, trp[:, 0:RT])
            # 3 layers, per out-chunk psum [128, 400]
            h1 = dec.tile([128, 4, RT], F32R, tag='dh1', name=f'dh1_{ti}')
            h2 = dec.tile([128, 4, RT], F32R, tag='dh2', name=f'dh2_{ti}')
            for m in range(4):
                pt = psD.tile([128, RT], F32, tag='dl', name=f'dl1_{ti}_{m}')
                nc.tensor.matmul(pt, d1l[:, m, :], latT, start=True, stop=False)
                nc.tensor.matmul(pt, d1z[:, m, :], zT, start=False, stop=False)
                nc.tensor.matmul(pt, d1x[0:1, m * 128:(m + 1) * 128],
                                 xs[0:1, r0:r0 + RT], start=False, stop=True)
                nc.scalar.activation(h1[:, m, :], pt, RELU,
                                     bias=dbias[:, m, 0:1])
            for m in range(4):
                pt = psD.tile([128, RT], F32, tag='dl', name=f'dl2_{ti}_{m}')
                for k in range(4):
                    nc.tensor.matmul(pt, d2[:, k, m, :], h1[:, k, :],
                                     start=(k == 0), stop=(k == 3))
                nc.scalar.activation(h2[:, m, :], pt, RELU,
                                     bias=dbias[:, m, 1:2])
            for m in range(4):
                pt = psD.tile([128, RT], F32, tag='dl', name=f'dl3_{ti}_{m}')
                for k in range(4):
                    nc.tensor.matmul(pt, d3[:, k, m, :], h2[:, k, :],
                                     start=(k == 0), stop=(k == 3))
                ot = dec.tile([128, RT], F32, tag='ot', name=f'ot_{ti}_{m}')
                nc.scalar.activation(ot, pt, RELU, bias=dbias[:, m, 2:3])
                nc.sync.dma_start(out=out_d[m][:, r0:r0 + RT], in_=ot)

        # ---- main schedule: ODE steps with decode tiles interleaved ----
        next_tile = 0
        for s in range(NST):
            h1kb, kbt = {}, {}
            for h in range(NH):
                h1kb[h] = psA.tile([128, 4 * NB], F32, tag=f'h1_{h}',
                                   name=f'h1kb_{h}_{s}')
                kbt[h] = psA.tile([128, 6 * NB], F32, tag=f'kb_{h}',
                                  name=f'kb_{h}_{s}')

            def mk_kb(h):
                def kb_ap(j, _t=kbt[h]):
                    return _t[:, j * NB:(j + 1) * NB]
                return kb_ap
            for i in range(6):
                for h in range(NH):
                    h2p = psA.tile([128, 4 * NB], F32, tag=f'h2_{h}',
                                   name=f'h2_{h}_{s}_{i}')
                    emit_stage(h, s, i, h1kb[h], h2p, mk_kb(h))
            for h in range(NH):
                emit_vupdate(h, s, mk_kb(h))
            while next_tile < NTILE and maxinds[next_tile] <= s + 1:
                emit_decode_tile(next_tile, maxinds[next_tile])
                next_tile += 1
        while next_tile < NTILE:
            emit_decode_tile(next_tile, maxinds[next_tile])
            next_tile += 1

    nc.compile()
    _BUILD_CACHE[key] = nc
    return nc


def _prep(x, z, initial_t, ode_W1, ode_b1, ode_W2, ode_b2, ode_W3, ode_b3,
          dec_W1, dec_b1, dec_W2, dec_b2, dec_W3, dec_b3):
    """All host-side preprocessing. Returns (in_maps, postprocess_info)."""
    x = np.asarray(x, np.float32)
    z = np.asarray(z, np.float32)
    x0 = np.float32(np.asarray(initial_t).reshape(-1)[0])
    xi = x.reshape(B, N)
    xsort = np.concatenate([np.full((B, 1), x0, np.float32), xi], axis=1)
    times, inv = np.unique(xsort, return_inverse=True)
    assert times.size == NST + 1, f'unique times {times.size} != {NST + 1}'
    ind = inv.reshape(B, N + 1)[:, 1:].astype(np.int64)   # [B, N] in [1,100]
    assert ind.min() >= 1
    dts = (times[1:] - times[:-1]).astype(np.float32)

    # per-stage effective L1 bias (t-term + b3 feedthrough)
    w1t = ode_W1[ZD]                                   # [512]
    b3w1l = (ode_b3.astype(np.float64) @ ode_W1[:LD].astype(np.float64))
    b1eff = np.zeros((NST, 6, HD), np.float32)
    for s in range(NST):
        for i in range(6):
            t_si = np.float32(times[s]) + np.float32(dts[s]) * np.float32(C_[i])
            sa = float(np.sum([np.float32(dts[s]) * np.float32(a)
                               for a in A_[i]])) if A_[i] else 0.0
            b1eff[s, i] = (ode_b1.astype(np.float64) + float(t_si) *
                           w1t.astype(np.float64) + sa * b3w1l)
    sb = float(np.sum([np.float32(b) for b in B_]))
    db3 = np.outer(dts.astype(np.float64) * sb,
                   ode_b3.astype(np.float64)).astype(np.float32)  # [NST, 128]

    def lhsT_tiles(w, kt, mt):
        # w [kt*128, mt*128] -> [128, kt, mt, 128]
        return np.ascontiguousarray(
            w.reshape(kt, 128, mt, 128).transpose(1, 0, 2, 3)).astype(np.float32)

    w1l_h = lhsT_tiles(ode_W1[:128], 1, 4).reshape(128, 4, 128)
    w2_h = lhsT_tiles(ode_W2, 4, 4)
    w3_h = lhsT_tiles(ode_W3, 4, 1).reshape(128, 4, 128)
    d1l_h = lhsT_tiles(dec_W1[1:129], 1, 4).reshape(128, 4, 128)
    d1z_h = lhsT_tiles(dec_W1[129:257], 1, 4).reshape(128, 4, 128)
    d1x_h = dec_W1[0:1].astype(np.float32)             # [1, 512]
    d2_h = lhsT_tiles(dec_W2, 4, 4)
    d3_h = lhsT_tiles(dec_W3, 4, 4)
    dbias_h = np.stack([dec_b1.reshape(4, 128).T, dec_b2.reshape(4, 128).T,
                        dec_b3.reshape(4, 128).T], axis=2).astype(np.float32)

    b1res_h = np.ascontiguousarray(
        b1eff.reshape(NST * 6, 4, 128).transpose(2, 0, 1)).astype(np.float32)
    b2rep_h = np.ascontiguousarray(ode_b2.reshape(4, 128).T).astype(np.float32)
    db3_h = np.ascontiguousarray(db3.T).astype(np.float32)  # [128, NST]

    # z static decode contribution is via gather; c1 is the ODE one
    c1_all = (z[:, LD:].astype(np.float64) @
              ode_W1[LD:ZD].astype(np.float64)).astype(np.float32)  # [B, 512]

    in_maps = []
    tiles_info = []
    for c in range(NCORE):
        sl = slice(c * BL, (c + 1) * BL)
        zc = z[sl]
        ind_c = ind[sl].reshape(-1)                     # [12800]
        order = np.argsort(ind_c, kind='stable')
        ind_sorted = ind_c[order]
        b_sorted = (order // N).astype(np.int64)
        gvals = (ind_sorted * BL + b_sorted).astype(np.int16)
        gidx_h = np.zeros((128, NTILE * 32), np.int16)
        maxind_c = []
        for ti in range(NTILE):
            seg = gvals[ti * RT:(ti + 1) * RT]
            for j in range(RT):
                gidx_h[j % 16::16, ti * 32 + j // 16] = seg[j]
            maxind_c.append(int(ind_sorted[ti * RT:(ti + 1) * RT].max()))
        xs_h = xi[sl].reshape(-1)[order].astype(np.float32)[None, :]
        c1T = np.ascontiguousarray(
            c1_all[sl].reshape(BL, 4, 128).transpose(2, 1, 0)).astype(np.float32)
        in_maps.append({
            'w1l': w1l_h, 'w2': w2_h, 'w3': w3_h, 'b1res': b1res_h,
            'b2rep': b2rep_h, 'db3': db3_h, 'c1': c1T,
            'vlt0': np.ascontiguousarray(zc[:, :LD].T).astype(np.float32),
            'zl': np.ascontiguousarray(zc[:, :LD]).astype(np.float32),
            'zz': np.ascontiguousarray(zc[:, LD:]).astype(np.float32),
            'd1l': d1l_h, 'd1z': d1z_h, 'd1x': d1x_h, 'd2': d2_h, 'd3': d3_h,
            'dbias': dbias_h, 'xs': xs_h, 'gidx': gidx_h,
        })
        tiles_info.append((order, maxind_c))
    # all cores must share one schedule: use elementwise max over cores
    maxinds = [max(tiles_info[c][1][t] for c in range(NCORE))
               for t in range(NTILE)]
    orders = [tiles_info[c][0] for c in range(NCORE)]
    return in_maps, maxinds, orders, [float(d) for d in dts]


def _postprocess(results, orders):
    out = np.empty((B, N, HD), np.float32)
    for c in range(NCORE):
        o = results[c]['outT']                          # [4, 128, ROWS]
        flat = np.ascontiguousarray(o.transpose(2, 0, 1)).reshape(ROWS, HD)
        unsorted = np.empty_like(flat)
        unsorted[orders[c]] = flat
        out[c * BL:(c + 1) * BL] = unsorted.reshape(BL, N, HD)
    return out


def kernel(**inputs):
    in_maps, maxinds, orders, dts = _prep(**inputs)
    nc = _build(maxinds, dts)
    from concourse.bass_utils import run_bass_kernel_spmd
    res = run_bass_kernel_spmd(nc, in_maps, list(range(NCORE)))
    return _postprocess(res.results, orders)
